# revision 20
# baseline (speedup 1.0000x reference)
"""Single-head attention (B=4, S=4096, F=H=1024) on 8 TRN2 NeuronCores.

Sharding: core = 2*b + h owns batch b, sequence-half h (rows h*2048 ..
(h+1)*2048). Each core projects K/Q/V only for its OWN 2048 rows, then the
two cores of a batch exchange K^T and V with pair-wise AllGathers (2-core
replica groups), slab-granular so comm hides behind compute.

Precision scheme (validated offline against the seeded reference inputs,
measured on HW: rel-err 1.879e-2 < 2e-2 gate, bit-stable across runs):
  - Q projection in bf16 (adding it to fp8 would push total error to
    2.33e-2, over the gate).  K and V projections in fp8 DoubleRow (x and
    W both e4m3 at scale 16, host-quantized).  V has NO bias -- since
    softmax weights sum to 1, out = sum_k w_k V0[k] + bv, so bv is added
    at the output stage instead.  PSUM->fp8 conversions alternate between
    the scalar (activation) and vector (tensor_scalar) engines so neither
    paces the tensor engine.
  - Q^T, K^T, V stored as e4m3 fp8 at scale 16; the scores matmul and the
    attention*V matmul run in fp8 DoubleRow perf mode (256-row contraction
    per instruction, 2x bf16 FLOP rate).
  - P = exp(s) has ~2.7% fp8 quantization noise; instead store
    P' = (P - C)*16 in fp8 (C ~ E[P]) -- a 3x smaller quantization target --
    and add back the rank-1 correction C * colsum(V_true) in the output
    stage. colsum(V_true) = colsum(x) @ Wv (bf16 weights, no bias) is
    precomputed on HOST and shipped as a broadcast [128, H] bf16 tile
    (csfull, pre-scaled by 256*C); likewise bvfull = broadcast bv.
    Denominator = sum_k P'/256 + S*C via a DoubleRow ones-column matmul and
    a scalar add before the reciprocal.
  - Output stage per 128-row tile: 2 DVE ops per 512-col half:
      acc = o_psum + csfull;  out = acc*recip + bvfull   (scalar_tensor_tensor)
    Output stored bf16 on device (halves the 8MB output stream; ~0.1%
    incoherent rounding, total 1.874e-2) and upcast to f32 on host.

Per-core math:
  x^T (own half) passed pre-transposed bf16 from host: [F=1024, 2048];
  quantized on-device to e4m3 (scale 16) for the V projection stationary.
  K^T[h,s] = fp8(16*(sum_f Wk[f,h] x^T[f,s] + bk))   (activation scale+bias)
  Q^T likewise, resident in SBUF. V0[s,h] = fp8(16*(x@Wv)), fp8 DR matmul.
  S^T[k,q] = sum_h K^T[h,k] Q^T[h,q]  (fp8 DoubleRow, 4 instr per 512 q)
  P = exp(S^T/8192) -> bf16;  P' = (P - C)*16 -> fp8 (DVE 2-op, pair-packed)
  out[q,:] = (P'^T V0/256 + C colsum(V0)) / (sum_k P'/256 + S*C) + bv
"""

import numpy as np
import ml_dtypes

# bass_utils' trace path imports antenv.axon_hooks, which some images lack;
# provide a no-op fallback so an externally-set BASS_TRACE cannot crash us.
try:
    import antenv.axon_hooks  # noqa: F401
except Exception:  # pragma: no cover
    try:
        import sys as _sys
        import types as _types

        import antenv as _antenv

        _m = _types.ModuleType("antenv.axon_hooks")
        _m.set_axon_ntff_profile_hook = lambda h: None
        _m.get_axon_ntff_profile_hook = lambda: None
        _sys.modules["antenv.axon_hooks"] = _m
        _antenv.axon_hooks = _m
    except Exception:
        pass

import concourse.bass as bass  # noqa: F401  (registers engine types)
import concourse.mybir as mybir
import concourse.tile as tile
from concourse import bacc
from concourse.bass_utils import run_bass_kernel_spmd

BF16 = mybir.dt.bfloat16
F8 = mybir.dt.float8e4
F32 = mybir.dt.float32
AF = mybir.ActivationFunctionType
DR = mybir.MatmulPerfMode.DoubleRow
ALU = mybir.AluOpType

B, S, F, H = 4, 4096, 1024, 1024
QH = S // 2  # rows owned per core
FC = F // 128  # 8 feature chunks
HC = H // 128  # 8 hidden chunks
N_CORES = 8
QS = 16.0  # fp8 scale for q/k/v/p'
CMEAN = 1.0568  # ~ E[exp(score)] for these inputs; any value is *correct*
EXP_SCALE = 1.0 / (32.0 * QS * QS)  # scores psum carries 256x
DEN_ADD = QS * QS * S * CMEAN  # add to den psum before reciprocal
PAIRS = [[0, 1], [2, 3], [4, 5], [6, 7]]

# key-chunk processing order: slab-0-dependent chunks (cols 0:1024 of each
# half) first, then slab-1 chunks.  k = half*16 + kk, slab = kk//8.
# Adjacent pairs share a gather-group g = slab*2+half and j parity (even,odd),
# which is exactly the DoubleRow pairing used in the AV matmul.
K_ORDER = (
    list(range(0, 8)) + list(range(16, 24)) + list(range(8, 16)) + list(range(24, 32))
)

_NC_CACHE = None


def _build_nc():
    nc = bacc.Bacc("TRN2", target_bir_lowering=False, debug=False)

    xt_ext = nc.declare_dram_parameter("xt", [128, 2, FC, 1024], BF16, isOutput=False)
    wq_ext = nc.declare_dram_parameter("wq", [128, FC, H], BF16, isOutput=False)
    wk8_ext = nc.declare_dram_parameter("wk8", [128, 4, FC, 256], F8, isOutput=False)
    x8t_ext = nc.declare_dram_parameter("x8t", [128, 2, 2, FC, 512], F8, isOutput=False)
    wv8_ext = nc.declare_dram_parameter("wv8", [128, FC, H], F8, isOutput=False)
    bqt_ext = nc.declare_dram_parameter("bqt", [128, HC], F32, isOutput=False)
    bkt_ext = nc.declare_dram_parameter("bkt", [128, HC], F32, isOutput=False)
    csf_ext = nc.declare_dram_parameter("csf", [128, H], BF16, isOutput=False)
    bvf_ext = nc.declare_dram_parameter("bvf", [128, H], BF16, isOutput=False)
    out_ext = nc.declare_dram_parameter("out", [QH, H], BF16, isOutput=True)


    with tile.TileContext(nc) as tc:
        with (
            tc.tile_pool(name="const", bufs=1) as constp,
            tc.tile_pool(name="qtres", bufs=1) as qtpool,
            tc.tile_pool(name="spill", bufs=1, space="DRAM") as dramp,
        ):
            cones8 = constp.tile([128, 2, 1], F8, tag="cones8", name="cones8")
            nc.vector.memset(cones8[:], QS)
            bqt = constp.tile([128, HC], F32, tag="bqt", name="bqt")
            bkt = constp.tile([128, HC], F32, tag="bkt", name="bkt")
            csfull = constp.tile([128, H], BF16, tag="csfull", name="csfull")
            bvfull = constp.tile([128, H], BF16, tag="bvfull", name="bvfull")

            # per-slab own spills + gathered pair buffers (plain Local DRAM)
            kt_own = [
                dramp.tile([HC, 128, 1024], F8, tag=f"kto{s}", name=f"kt_own{s}")
                for s in range(2)
            ]
            v_own = [
                dramp.tile([1024, H], F8, tag=f"vo{s}", name=f"v_own{s}")
                for s in range(2)
            ]
            kt_gath = [
                dramp.tile([2, HC, 128, 1024], F8, tag=f"ktg{s}", name=f"kt_gath{s}")
                for s in range(2)
            ]
            v_gath = [
                dramp.tile([2, 1024, H], F8, tag=f"vg{s}", name=f"v_gath{s}")
                for s in range(2)
            ]

            qt_res = qtpool.tile([128, HC, QH], F8, tag="qtres", name="qt_res")
            # g = slab*2 + half; prefetched during phase A's Q projection
            vbig = [
                qtpool.tile([128, 8, H], F8, tag=f"vb{g}", name=f"vbig{g}")
                for g in range(4)
            ]
            ktbig = [
                qtpool.tile([128, HC, 1024], F8, tag=f"kb{g}", name=f"ktbig{g}")
                for g in range(4)
            ]

            def pair_gather(dst, src):
                nc.gpsimd.collective_compute(
                    "AllGather", mybir.AluOpType.bypass, replica_groups=PAIRS,
                    ins=[src.opt()], outs=[dst.opt()],
                )

            # ---------- Phase A: own-half projections in one x^T pass ----------
            with (
                tc.tile_pool(name="wp", bufs=1) as wp,
                tc.tile_pool(name="xp", bufs=2) as xp,
                tc.tile_pool(name="x8p", bufs=2) as x8p,
                tc.tile_pool(name="stage", bufs=2) as stp,
                tc.tile_pool(name="psA", bufs=8, space="PSUM") as psA,
            ):
                wk8_sb = wp.tile([128, FC, H], F8, tag="wk8", name="wk8_sb")
                wq_sb = wp.tile([128, FC, H], BF16, tag="wq", name="wq_sb")
                wv8_sb = wp.tile([128, FC, H], F8, tag="wv8", name="wv8_sb")
                # startup-critical loads, finest first: the first K-proj psum
                # group needs wk8 cols 0:256 (256KB) + x8 cols 0:512 (512KB).
                # All bulk inputs are host-relaid partition-major so each DMA
                # is 128 descriptors of 2-16KB.  Small/late tiles go on the
                # gpsimd queue so they don't serialize these.
                nc.sync.dma_start(wk8_sb[:, :, 0:256], wk8_ext[:, 0])

                xts_l, x8t_l = [], []
                for sp in range(QH // 1024):
                    x8t = x8p.tile([128, FC, 1024], F8, tag="x8t", name=f"x8t{sp}")
                    x8t_l.append(x8t)
                nc.sync.dma_start(x8t_l[0][:, :, 0:512], x8t_ext[:, 0, 0])
                nc.gpsimd.dma_start(bkt[:], bkt_ext[:])
                for piece in range(1, 4):
                    c0 = piece * 256
                    nc.sync.dma_start(wk8_sb[:, :, c0 : c0 + 256], wk8_ext[:, piece])
                nc.sync.dma_start(x8t_l[0][:, :, 512:1024], x8t_ext[:, 0, 1])
                nc.sync.dma_start(x8t_l[1][:, :, 0:512], x8t_ext[:, 1, 0])
                nc.sync.dma_start(x8t_l[1][:, :, 512:1024], x8t_ext[:, 1, 1])
                # wv8 next (V-proj ~45us in); bf16 x^T + wq only feed Q-proj
                # (~70us in) so they stream last.
                nc.sync.dma_start(wv8_sb[:], wv8_ext[:])
                for sp in range(QH // 1024):
                    xts = xp.tile([128, FC, 1024], BF16, tag="xts", name=f"xts{sp}")
                    xts_l.append(xts)
                    nc.sync.dma_start(xts[:], xt_ext[:, sp])
                nc.sync.dma_start(wq_sb[:], wq_ext[:])
                nc.gpsimd.dma_start(bqt[:], bqt_ext[:])
                nc.gpsimd.dma_start(csfull[:], csf_ext[:])
                nc.gpsimd.dma_start(bvfull[:], bvf_ext[:])

                # K^T both slabs first, so both pair-gathers start early.
                # Key-half-major order: the second x^T half-slab DMA streams
                # behind the first half's 13us of matmuls instead of stalling
                # the per-hh ps0/ps1 interleave at startup.
                for sp in range(QH // 1024):
                    x8t = x8t_l[sp]
                    for kh in range(2):
                        kq = slice(kh * 512, kh * 512 + 512)
                        kst = stp.tile(
                            [128, HC, 512], F8, tag=f"kst{sp}{kh}", bufs=1,
                            name=f"ks{sp}_{kh}",
                        )
                        # hh pairs alternate two PSUM banks: back-to-back
                        # accumulation into one bank costs ~45ns/matmul extra
                        for hp in range(HC // 2):
                            ps0 = psA.tile(
                                [128, 512], F32, tag="psA", name=f"pk0_{sp}_{kh}_{hp}"
                            )
                            ps1 = psA.tile(
                                [128, 512], F32, tag="psA", name=f"pk1_{sp}_{kh}_{hp}"
                            )
                            for f2 in range(FC // 2):
                                f8s = slice(2 * f2, 2 * f2 + 2)
                                nc.tensor.matmul(
                                    ps0[:],
                                    wk8_sb[:, f8s, (2 * hp) * 128 : (2 * hp + 1) * 128],
                                    x8t[:, f8s, kq],
                                    start=(f2 == 0), stop=(f2 == FC // 2 - 1),
                                    perf_mode=DR,
                                )
                                nc.tensor.matmul(
                                    ps1[:],
                                    wk8_sb[:, f8s, (2 * hp + 1) * 128 : (2 * hp + 2) * 128],
                                    x8t[:, f8s, kq],
                                    start=(f2 == 0), stop=(f2 == FC // 2 - 1),
                                    perf_mode=DR,
                                )
                            # psum = 256*K0; store fp8(16*(K0+bk)) = ps/16 + 16*bk
                            # split across scalar+DVE so neither paces the
                            # tensor engine (each conv ~0.8us vs 1.7us matmul)
                            nc.scalar.activation(
                                kst[:, 2 * hp, :], ps0[:], AF.Identity,
                                bias=bkt[:, 2 * hp : 2 * hp + 1], scale=1.0 / QS,
                            )
                            nc.vector.tensor_scalar(
                                kst[:, 2 * hp + 1, :], ps1[:], 1.0 / QS,
                                bkt[:, 2 * hp + 1 : 2 * hp + 2],
                                ALU.mult, ALU.add,
                            )
                        nc.gpsimd.dma_start(
                            kt_own[sp][:, :, kq].rearrange("c p q -> p c q"), kst[:]
                        )
                    pair_gather(kt_gath[sp], kt_own[sp])

                # V both slabs: fp8 DoubleRow (x8t stationary, wv8 moving),
                # no bias -- bv is folded into the output stage.
                for sp in range(QH // 1024):
                    x8t = x8t_l[sp]
                    vst = stp.tile([128, 8, H], F8, tag=f"vst{sp}", bufs=1, name=f"vst{sp}")
                    for sc in range(8):
                        ps0 = psA.tile([128, 512], F32, tag="psA", name=f"pv0_{sp}_{sc}")
                        ps1 = psA.tile([128, 512], F32, tag="psA", name=f"pv1_{sp}_{sc}")
                        for f2 in range(FC // 2):
                            lhs = x8t[:, 2 * f2 : 2 * f2 + 2, sc * 128 : (sc + 1) * 128]
                            nc.tensor.matmul(
                                ps0[:], lhs, wv8_sb[:, 2 * f2 : 2 * f2 + 2, 0:512],
                                start=(f2 == 0), stop=(f2 == FC // 2 - 1),
                                perf_mode=DR,
                            )
                            nc.tensor.matmul(
                                ps1[:], lhs, wv8_sb[:, 2 * f2 : 2 * f2 + 2, 512:1024],
                                start=(f2 == 0), stop=(f2 == FC // 2 - 1),
                                perf_mode=DR,
                            )
                        # psum = 256*V0; store 16*V0 as e4m3 (split DVE/scalar)
                        nc.vector.tensor_scalar_mul(vst[:, sc, 0:512], ps0[:], 1.0 / QS)
                        nc.scalar.activation(
                            vst[:, sc, 512:1024], ps1[:], AF.Identity, scale=1.0 / QS
                        )
                    nc.gpsimd.dma_start(
                        v_own[sp][:].rearrange("(c p) h -> p c h", p=128), vst[:]
                    )
                    pair_gather(v_gath[sp], v_own[sp])

                # prefetch gathered K^T/V into phase-B residents while Q proj
                # still runs (kvpool lives in the outer scope)
                for g in range(4):
                    slab, half = g // 2, g % 2
                    nc.sync.dma_start(
                        ktbig[g][:],
                        kt_gath[slab][half].rearrange("c p k -> p c k"),
                    )
                    nc.sync.dma_start(
                        vbig[g][:],
                        v_gath[slab][half].rearrange("(c p) h -> p c h", p=128),
                    )

                # Q^T both slabs -> resident SBUF (fp8 at scale QS)
                for sp in range(QH // 1024):
                    xts = xts_l[sp]
                    base = sp * 1024
                    for hh in range(HC):
                        ps0 = psA.tile([128, 512], F32, tag="psA", name=f"pq0_{sp}_{hh}")
                        ps1 = psA.tile([128, 512], F32, tag="psA", name=f"pq1_{sp}_{hh}")
                        for f in range(FC):
                            lhs = wq_sb[:, f, hh * 128 : (hh + 1) * 128]
                            nc.tensor.matmul(
                                ps0[:], lhs, xts[:, f, 0:512],
                                start=(f == 0), stop=(f == FC - 1),
                            )
                            nc.tensor.matmul(
                                ps1[:], lhs, xts[:, f, 512:1024],
                                start=(f == 0), stop=(f == FC - 1),
                            )
                        bias = bqt[:, hh : hh + 1]
                        nc.scalar.activation(
                            qt_res[:, hh, base : base + 512], ps0[:],
                            AF.Identity, bias=bias, scale=QS,
                        )
                        nc.vector.tensor_scalar(
                            qt_res[:, hh, base + 512 : base + 1024], ps1[:],
                            QS, bias, ALU.mult, ALU.add,
                        )

            # ---------- Phase B: attention, 1024 query rows per tile ----------
            with (
                tc.tile_pool(name="expp", bufs=2) as expp,
                tc.tile_pool(name="pexpp", bufs=4) as pexpp,
                tc.tile_pool(name="obp", bufs=3) as obp,
                tc.tile_pool(name="psS", bufs=3, space="PSUM") as psS,
                tc.tile_pool(name="psO", bufs=2, space="PSUM") as psO,
            ):
                for qt in range(QH // 1024):
                    qbase = qt * 1024
                    # scores + exp + P' quant, one key-chunk at a time
                    exps = {}
                    for pos, k in enumerate(K_ORDER):
                        i, par = pos // 2, pos % 2
                        half, kk = k // 16, k % 16
                        slab, kk8 = kk // 8, kk % 8
                        g = slab * 2 + half
                        if par == 0:
                            exps[i] = expp.tile(
                                [128, 2, 1024], F8, tag=f"e{i}", name=f"e{qt}_{i}"
                            )
                        ps0 = psS.tile([128, 512], F32, tag="psS", name=f"pS0_{qt}_{k}")
                        ps1 = psS.tile([128, 512], F32, tag="psS", name=f"pS1_{qt}_{k}")
                        kslice = slice(kk8 * 128, (kk8 + 1) * 128)
                        for hp in range(HC // 2):
                            nc.tensor.matmul(
                                ps0[:], ktbig[g][:, 2 * hp : 2 * hp + 2, kslice],
                                qt_res[:, 2 * hp : 2 * hp + 2, qbase : qbase + 512],
                                start=(hp == 0), stop=(hp == HC // 2 - 1),
                                perf_mode=DR,
                            )
                        for hp in range(HC // 2):
                            nc.tensor.matmul(
                                ps1[:], ktbig[g][:, 2 * hp : 2 * hp + 2, kslice],
                                qt_res[:, 2 * hp : 2 * hp + 2, qbase + 512 : qbase + 1024],
                                start=(hp == 0), stop=(hp == HC // 2 - 1),
                                perf_mode=DR,
                            )
                        pexp = pexpp.tile([128, 1024], BF16, tag="pexp", name=f"px{qt}_{k}")
                        nc.scalar.activation(pexp[:, 0:512], ps0[:], AF.Exp, scale=EXP_SCALE)
                        nc.scalar.activation(pexp[:, 512:1024], ps1[:], AF.Exp, scale=EXP_SCALE)
                        nc.vector.tensor_scalar(
                            exps[i][:, par, :], pexp[:], -CMEAN, QS,
                            mybir.AluOpType.add, mybir.AluOpType.mult,
                        )
                    # AV: fp8 DoubleRow over key-chunk pairs
                    for q1 in range(8):
                        qo = q1 * 128
                        o0 = psO.tile([128, 512], F32, tag="o0", name=f"o0_{qt}_{q1}")
                        o1 = psO.tile([128, 512], F32, tag="o1", name=f"o1_{qt}_{q1}")
                        osum = psO.tile(
                            [128, 1], F32, tag="osum", bufs=1, name=f"os{qt}_{q1}"
                        )
                        for t in range(16):
                            k0 = K_ORDER[2 * t]
                            half, kk = k0 // 16, k0 % 16
                            slab, j = kk // 8, kk % 8
                            g = slab * 2 + half
                            lhs = exps[t][:, :, qo : qo + 128]
                            first, last = t == 0, t == 15
                            nc.tensor.matmul(
                                o0[:], lhs, vbig[g][:, j : j + 2, 0:512],
                                start=first, stop=last, perf_mode=DR,
                            )
                            nc.tensor.matmul(
                                o1[:], lhs, vbig[g][:, j : j + 2, 512:1024],
                                start=first, stop=last, perf_mode=DR,
                            )
                            nc.tensor.matmul(
                                osum[:], lhs, cones8[:], start=first, stop=last,
                                perf_mode=DR,
                            )
                        den = obp.tile([128, 1], F32, tag="den", name=f"dn{qt}_{q1}")
                        nc.vector.tensor_scalar_add(den[:], osum[:], DEN_ADD)
                        recip = obp.tile([128, 1], F32, tag="recip", name=f"rc{qt}_{q1}")
                        nc.vector.reciprocal(recip[:], den[:])
                        outsb = obp.tile([128, H], BF16, tag="outsb", name=f"ou{qt}_{q1}")
                        acc = obp.tile([128, H], F32, tag="acc", name=f"ac{qt}_{q1}")
                        row = qbase + qo
                        for hf, ps in ((0, o0), (1, o1)):
                            cs = slice(hf * 512, hf * 512 + 512)
                            nc.vector.tensor_tensor(
                                acc[:, cs], ps[:], csfull[:, cs], ALU.add
                            )
                            nc.vector.scalar_tensor_tensor(
                                outsb[:, cs], acc[:, cs], recip[:], bvfull[:, cs],
                                ALU.mult, ALU.add,
                            )
                        nc.sync.dma_start(out_ext[row : row + 128, :], outsb[:])

    nc.compile()
    return nc


def _get_nc():
    global _NC_CACHE
    if _NC_CACHE is None:
        _NC_CACHE = _build_nc()
    return _NC_CACHE


def _make_in_maps(x, Wq, bq, Wk, bk, Wv, bv):
    bf16 = ml_dtypes.bfloat16
    f8 = ml_dtypes.float8_e4m3fn
    def pcmajor(w):  # [F, H] -> [128, FC, H] partition-major
        return np.ascontiguousarray(w.reshape(FC, 128, H).transpose(1, 0, 2))

    wq_b = pcmajor(np.asarray(Wq, np.float32).astype(bf16))
    wk_b32 = np.asarray(Wk, np.float32).astype(bf16).astype(np.float32)
    # [F, H] -> [128, 4 col-pieces, FC, 256]
    wk8 = np.ascontiguousarray(
        (QS * wk_b32).astype(f8).reshape(FC, 128, 4, 256).transpose(1, 2, 0, 3)
    )
    wv_b32 = np.asarray(Wv, np.float32).astype(bf16).astype(np.float32)
    wv8 = pcmajor((QS * wv_b32).astype(f8))
    # activation computes f(scale*x + bias) with scale=QS, so pre-scale biases
    bqt = np.ascontiguousarray(QS * np.asarray(bq, np.float32).reshape(HC, 128).T)
    bkt = np.ascontiguousarray(QS * np.asarray(bk, np.float32).reshape(HC, 128).T)
    bvf = np.ascontiguousarray(
        np.broadcast_to(np.asarray(bv, np.float32).astype(bf16).reshape(1, H), (128, H))
    )
    x = np.asarray(x, np.float32)
    # host-side rank-1 correction row: 256*C*colsum(V0_true) per batch,
    # V0_true = x @ bf16(Wv) (no bias)
    in_maps = []
    for core in range(N_CORES):
        b, h = core // 2, core % 2
        xt_flat = np.ascontiguousarray(x[b, h * QH : (h + 1) * QH].T).astype(bf16)
        x8t_flat = (QS * xt_flat.astype(np.float32)).astype(f8)
        # [F, QH] -> [128, slab 2, FC, 1024] partition-major
        xt = np.ascontiguousarray(
            xt_flat.reshape(FC, 128, 2, 1024).transpose(1, 2, 0, 3)
        )
        x8t = np.ascontiguousarray(
            x8t_flat.reshape(FC, 128, 2, 2, 512).transpose(1, 2, 3, 0, 4)
        )
        csrow = (QS * QS * CMEAN) * (x[b].sum(axis=0) @ wv_b32)
        csf = np.ascontiguousarray(
            np.broadcast_to(csrow.astype(np.float32).astype(bf16).reshape(1, H), (128, H))
        )
        in_maps.append(
            {
                "xt": xt,
                "x8t": x8t,
                "wq": wq_b,
                "wk8": wk8,
                "wv8": wv8,
                "bqt": bqt,
                "bkt": bkt,
                "csf": csf,
                "bvf": bvf,
            }
        )
    return in_maps


def run_on_hw(inputs, trace=False, tmpdir=None):
    """Returns (full_output, BassKernelResults)."""
    nc = _get_nc()
    in_maps = _make_in_maps(**inputs)
    res = run_bass_kernel_spmd(
        nc, in_maps, core_ids=list(range(N_CORES)), trace=trace, tmpdir=tmpdir
    )
    out = np.empty((B, S, H), np.float32)
    for core in range(N_CORES):
        b, h = core // 2, core % 2
        out[b, h * QH : (h + 1) * QH] = res.results[core]["out"].astype(np.float32)
    return out, res


def kernel(x, Wq, bq, Wk, bk, Wv, bv):
    out, _ = run_on_hw(
        {"x": x, "Wq": Wq, "bq": bq, "Wk": Wk, "bk": bk, "Wv": Wv, "bv": bv}
    )
    return out


# revision 21
# speedup vs baseline: 1.0545x; 1.0545x over previous
"""Single-head attention (B=4, S=4096, F=H=1024) on 8 TRN2 NeuronCores.

Sharding: core = 2*b + h owns batch b, sequence-half h (rows h*2048 ..
(h+1)*2048). Each core projects K/Q/V only for its OWN 2048 rows, then the
two cores of a batch exchange K^T and V with pair-wise AllGathers (2-core
replica groups), slab-granular so comm hides behind compute.

Precision scheme (validated offline against the seeded reference inputs,
measured on HW: rel-err 1.879e-2 < 2e-2 gate, bit-stable across runs):
  - Q projection in bf16 (adding it to fp8 would push total error to
    2.33e-2, over the gate).  K and V projections in fp8 DoubleRow (x and
    W both e4m3 at scale 16, host-quantized).  V has NO bias -- since
    softmax weights sum to 1, out = sum_k w_k V0[k] + bv, so bv is added
    at the output stage instead.  PSUM->fp8 conversions alternate between
    the scalar (activation) and vector (tensor_scalar) engines so neither
    paces the tensor engine.
  - Q^T, K^T, V stored as e4m3 fp8 at scale 16; the scores matmul and the
    attention*V matmul run in fp8 DoubleRow perf mode (256-row contraction
    per instruction, 2x bf16 FLOP rate).
  - P = exp(s) has ~2.7% fp8 quantization noise; instead store
    P' = (P - C)*16 in fp8 (C ~ E[P]) -- a 3x smaller quantization target --
    and add back the rank-1 correction C * colsum(V_true) in the output
    stage. colsum(V_true) = colsum(x) @ Wv (bf16 weights, no bias) is
    precomputed on HOST and shipped as a broadcast [128, H] bf16 tile
    (csfull, pre-scaled by 256*C); likewise bvfull = broadcast bv.
    Denominator = sum_k P'/256 + S*C via a DoubleRow ones-column matmul and
    a scalar add before the reciprocal.
  - Output stage per 128-row tile: 2 DVE ops per 512-col half:
      acc = o_psum + csfull;  out = acc*recip + bvfull   (scalar_tensor_tensor)
    Output stored bf16 on device (halves the 8MB output stream; ~0.1%
    incoherent rounding, total 1.874e-2) and upcast to f32 on host.

Per-core math:
  x^T (own half) passed pre-transposed bf16 from host: [F=1024, 2048];
  quantized on-device to e4m3 (scale 16) for the V projection stationary.
  K^T[h,s] = fp8(16*(sum_f Wk[f,h] x^T[f,s] + bk))   (activation scale+bias)
  Q^T likewise, resident in SBUF. V0[s,h] = fp8(16*(x@Wv)), fp8 DR matmul.
  S^T[k,q] = sum_h K^T[h,k] Q^T[h,q]  (fp8 DoubleRow, 4 instr per 512 q)
  P = exp(S^T/8192) -> bf16;  P' = (P - C)*16 -> fp8 (DVE 2-op, pair-packed)
  out[q,:] = (P'^T V0/256 + C colsum(V0)) / (sum_k P'/256 + S*C) + bv
"""

import numpy as np
import ml_dtypes

# bass_utils' trace path imports antenv.axon_hooks, which some images lack;
# provide a no-op fallback so an externally-set BASS_TRACE cannot crash us.
try:
    import antenv.axon_hooks  # noqa: F401
except Exception:  # pragma: no cover
    try:
        import sys as _sys
        import types as _types

        import antenv as _antenv

        _m = _types.ModuleType("antenv.axon_hooks")
        _m.set_axon_ntff_profile_hook = lambda h: None
        _m.get_axon_ntff_profile_hook = lambda: None
        _sys.modules["antenv.axon_hooks"] = _m
        _antenv.axon_hooks = _m
    except Exception:
        pass

import concourse.bass as bass  # noqa: F401  (registers engine types)
import concourse.mybir as mybir
import concourse.tile as tile
from concourse import bacc
from concourse.bass_utils import run_bass_kernel_spmd

BF16 = mybir.dt.bfloat16
F8 = mybir.dt.float8e4
F32 = mybir.dt.float32
AF = mybir.ActivationFunctionType
DR = mybir.MatmulPerfMode.DoubleRow
ALU = mybir.AluOpType

B, S, F, H = 4, 4096, 1024, 1024
QH = S // 2  # rows owned per core
FC = F // 128  # 8 feature chunks
HC = H // 128  # 8 hidden chunks
N_CORES = 8
QS = 16.0  # fp8 scale for q/k/v/p'
CMEAN = 1.0568  # ~ E[exp(score)] for these inputs; any value is *correct*
EXP_SCALE = 1.0 / (32.0 * QS * QS)  # scores psum carries 256x
DEN_ADD = QS * QS * S * CMEAN  # add to den psum before reciprocal
PAIRS = [[0, 1], [2, 3], [4, 5], [6, 7]]

# key-chunk processing order: slab-0-dependent chunks (cols 0:1024 of each
# half) first, then slab-1 chunks.  k = half*16 + kk, slab = kk//8.
# Adjacent pairs share a gather-group g = slab*2+half and j parity (even,odd),
# which is exactly the DoubleRow pairing used in the AV matmul.
K_ORDER = (
    list(range(0, 8)) + list(range(16, 24)) + list(range(8, 16)) + list(range(24, 32))
)

_NC_CACHE = None


def _build_nc():
    nc = bacc.Bacc("TRN2", target_bir_lowering=False, debug=False)

    xt_ext = nc.declare_dram_parameter("xt", [128, 2, FC, 1024], BF16, isOutput=False)
    wq_ext = nc.declare_dram_parameter("wq", [128, FC, H], BF16, isOutput=False)
    wk8_ext = nc.declare_dram_parameter("wk8", [128, 4, FC, 256], F8, isOutput=False)
    x8t_ext = nc.declare_dram_parameter("x8t", [128, 2, 2, FC, 512], F8, isOutput=False)
    wv8_ext = nc.declare_dram_parameter("wv8", [128, FC, H], F8, isOutput=False)
    bqt_ext = nc.declare_dram_parameter("bqt", [128, HC], F32, isOutput=False)
    bkt_ext = nc.declare_dram_parameter("bkt", [128, HC], F32, isOutput=False)
    csf_ext = nc.declare_dram_parameter("csf", [128, H], BF16, isOutput=False)
    bvf_ext = nc.declare_dram_parameter("bvf", [128, H], BF16, isOutput=False)
    out_ext = nc.declare_dram_parameter("out", [QH, H], BF16, isOutput=True)


    with tile.TileContext(nc) as tc:
        with (
            tc.tile_pool(name="const", bufs=1) as constp,
            tc.tile_pool(name="qtres", bufs=1) as qtpool,
            tc.tile_pool(name="spill", bufs=1, space="DRAM") as dramp,
        ):
            cones8 = constp.tile([128, 2, 1], F8, tag="cones8", name="cones8")
            nc.vector.memset(cones8[:], QS)
            bqt = constp.tile([128, HC], F32, tag="bqt", name="bqt")
            bkt = constp.tile([128, HC], F32, tag="bkt", name="bkt")
            csfull = constp.tile([128, H], BF16, tag="csfull", name="csfull")
            bvfull = constp.tile([128, H], BF16, tag="bvfull", name="bvfull")

            # per-slab own spills + gathered pair buffers (plain Local DRAM)
            kt_own = [
                dramp.tile([HC, 128, 1024], F8, tag=f"kto{s}", name=f"kt_own{s}")
                for s in range(2)
            ]
            v_own = [
                dramp.tile([1024, H], F8, tag=f"vo{s}", name=f"v_own{s}")
                for s in range(2)
            ]
            kt_gath = [
                dramp.tile([2, HC, 128, 1024], F8, tag=f"ktg{s}", name=f"kt_gath{s}")
                for s in range(2)
            ]
            v_gath = [
                dramp.tile([2, 1024, H], F8, tag=f"vg{s}", name=f"v_gath{s}")
                for s in range(2)
            ]

            qt_res = qtpool.tile([128, HC, QH], F8, tag="qtres", name="qt_res")
            # g = slab*2 + half; prefetched during phase A's Q projection
            vbig = [
                qtpool.tile([128, 8, H], F8, tag=f"vb{g}", name=f"vbig{g}")
                for g in range(4)
            ]
            ktbig = [
                qtpool.tile([128, HC, 1024], F8, tag=f"kb{g}", name=f"ktbig{g}")
                for g in range(4)
            ]

            def pair_gather(dst, src):
                nc.gpsimd.collective_compute(
                    "AllGather", mybir.AluOpType.bypass, replica_groups=PAIRS,
                    ins=[src.opt()], outs=[dst.opt()],
                )

            # ---------- Phase A: own-half projections in one x^T pass ----------
            with (
                tc.tile_pool(name="wp", bufs=1) as wp,
                tc.tile_pool(name="xp", bufs=2) as xp,
                tc.tile_pool(name="x8p", bufs=2) as x8p,
                tc.tile_pool(name="stage", bufs=2) as stp,
                tc.tile_pool(name="psA", bufs=8, space="PSUM") as psA,
            ):
                wk8_sb = wp.tile([128, FC, H], F8, tag="wk8", name="wk8_sb")
                wq_sb = wp.tile([128, FC, H], BF16, tag="wq", name="wq_sb")
                wv8_sb = wp.tile([128, FC, H], F8, tag="wv8", name="wv8_sb")
                # startup-critical loads, finest first: the first K-proj psum
                # group needs wk8 cols 0:256 (256KB) + x8 cols 0:512 (512KB).
                # All bulk inputs are host-relaid partition-major so each DMA
                # is 128 descriptors of 2-16KB.  Small/late tiles go on the
                # gpsimd queue so they don't serialize these.
                nc.sync.dma_start(wk8_sb[:, :, 0:256], wk8_ext[:, 0])

                xts_l, x8t_l = [], []
                for sp in range(QH // 1024):
                    x8t = x8p.tile([128, FC, 1024], F8, tag="x8t", name=f"x8t{sp}")
                    x8t_l.append(x8t)
                nc.sync.dma_start(x8t_l[0][:, :, 0:512], x8t_ext[:, 0, 0])
                nc.gpsimd.dma_start(bkt[:], bkt_ext[:])
                for piece in range(1, 4):
                    c0 = piece * 256
                    nc.sync.dma_start(wk8_sb[:, :, c0 : c0 + 256], wk8_ext[:, piece])
                nc.sync.dma_start(x8t_l[0][:, :, 512:1024], x8t_ext[:, 0, 1])
                nc.sync.dma_start(x8t_l[1][:, :, 0:512], x8t_ext[:, 1, 0])
                nc.sync.dma_start(x8t_l[1][:, :, 512:1024], x8t_ext[:, 1, 1])
                # wv8 next (V-proj ~45us in); bf16 x^T + wq only feed Q-proj
                # (~70us in) so they stream last.
                nc.sync.dma_start(wv8_sb[:], wv8_ext[:])
                for sp in range(QH // 1024):
                    xts = xp.tile([128, FC, 1024], BF16, tag="xts", name=f"xts{sp}")
                    xts_l.append(xts)
                    nc.sync.dma_start(xts[:], xt_ext[:, sp])
                nc.sync.dma_start(wq_sb[:], wq_ext[:])
                nc.gpsimd.dma_start(bqt[:], bqt_ext[:])
                nc.gpsimd.dma_start(csfull[:], csf_ext[:])
                nc.gpsimd.dma_start(bvfull[:], bvf_ext[:])

                # K^T both slabs first, so both pair-gathers start early.
                # Key-half-major order: the second x^T half-slab DMA streams
                # behind the first half's 13us of matmuls instead of stalling
                # the per-hh ps0/ps1 interleave at startup.
                for sp in range(QH // 1024):
                    x8t = x8t_l[sp]
                    for kh in range(2):
                        kq = slice(kh * 512, kh * 512 + 512)
                        kst = stp.tile(
                            [128, HC, 512], F8, tag=f"kst{sp}{kh}", bufs=1,
                            name=f"ks{sp}_{kh}",
                        )
                        # hh pairs alternate two PSUM banks: back-to-back
                        # accumulation into one bank costs ~45ns/matmul extra
                        for hp in range(HC // 2):
                            ps0 = psA.tile(
                                [128, 512], F32, tag="psA", name=f"pk0_{sp}_{kh}_{hp}"
                            )
                            ps1 = psA.tile(
                                [128, 512], F32, tag="psA", name=f"pk1_{sp}_{kh}_{hp}"
                            )
                            for f2 in range(FC // 2):
                                f8s = slice(2 * f2, 2 * f2 + 2)
                                nc.tensor.matmul(
                                    ps0[:],
                                    wk8_sb[:, f8s, (2 * hp) * 128 : (2 * hp + 1) * 128],
                                    x8t[:, f8s, kq],
                                    start=(f2 == 0), stop=(f2 == FC // 2 - 1),
                                    perf_mode=DR,
                                )
                                nc.tensor.matmul(
                                    ps1[:],
                                    wk8_sb[:, f8s, (2 * hp + 1) * 128 : (2 * hp + 2) * 128],
                                    x8t[:, f8s, kq],
                                    start=(f2 == 0), stop=(f2 == FC // 2 - 1),
                                    perf_mode=DR,
                                )
                            # psum = 256*K0; store fp8(16*(K0+bk)) = ps/16 + 16*bk
                            # split across scalar+DVE so neither paces the
                            # tensor engine (each conv ~0.8us vs 1.7us matmul)
                            nc.scalar.activation(
                                kst[:, 2 * hp, :], ps0[:], AF.Identity,
                                bias=bkt[:, 2 * hp : 2 * hp + 1], scale=1.0 / QS,
                            )
                            nc.vector.tensor_scalar(
                                kst[:, 2 * hp + 1, :], ps1[:], 1.0 / QS,
                                bkt[:, 2 * hp + 1 : 2 * hp + 2],
                                ALU.mult, ALU.add,
                            )
                        nc.gpsimd.dma_start(
                            kt_own[sp][:, :, kq].rearrange("c p q -> p c q"), kst[:]
                        )

                # both K gathers issue only after both slabs' spills are
                # queued, so a long-blocking collective can't delay the
                # slab-1 spill issue behind it on the gpsimd queue.
                pair_gather(kt_gath[0], kt_own[0])
                pair_gather(kt_gath[1], kt_own[1])

                # V both slabs: fp8 DoubleRow (x8t stationary, wv8 moving),
                # no bias -- bv is folded into the output stage.
                for sp in range(QH // 1024):
                    x8t = x8t_l[sp]
                    vst = stp.tile([128, 8, H], F8, tag=f"vst{sp}", bufs=1, name=f"vst{sp}")
                    for sc in range(8):
                        ps0 = psA.tile([128, 512], F32, tag="psA", name=f"pv0_{sp}_{sc}")
                        ps1 = psA.tile([128, 512], F32, tag="psA", name=f"pv1_{sp}_{sc}")
                        for f2 in range(FC // 2):
                            lhs = x8t[:, 2 * f2 : 2 * f2 + 2, sc * 128 : (sc + 1) * 128]
                            nc.tensor.matmul(
                                ps0[:], lhs, wv8_sb[:, 2 * f2 : 2 * f2 + 2, 0:512],
                                start=(f2 == 0), stop=(f2 == FC // 2 - 1),
                                perf_mode=DR,
                            )
                            nc.tensor.matmul(
                                ps1[:], lhs, wv8_sb[:, 2 * f2 : 2 * f2 + 2, 512:1024],
                                start=(f2 == 0), stop=(f2 == FC // 2 - 1),
                                perf_mode=DR,
                            )
                        # psum = 256*V0; store 16*V0 as e4m3 (split DVE/scalar)
                        nc.vector.tensor_scalar_mul(vst[:, sc, 0:512], ps0[:], 1.0 / QS)
                        nc.scalar.activation(
                            vst[:, sc, 512:1024], ps1[:], AF.Identity, scale=1.0 / QS
                        )
                    nc.gpsimd.dma_start(
                        v_own[sp][:].rearrange("(c p) h -> p c h", p=128), vst[:]
                    )
                    pair_gather(v_gath[sp], v_own[sp])

                # prefetch gathered K^T/V into phase-B residents while Q proj
                # still runs (kvpool lives in the outer scope)
                # all K^T prefetches first: the scores stream must never
                # sit in the queue behind a vbig DMA whose v-gather hasn't
                # fired yet (queue FIFO blocks on each DMA's sem wait).
                for g in range(4):
                    slab, half = g // 2, g % 2
                    nc.sync.dma_start(
                        ktbig[g][:],
                        kt_gath[slab][half].rearrange("c p k -> p c k"),
                    )
                for g in range(4):
                    slab, half = g // 2, g % 2
                    nc.sync.dma_start(
                        vbig[g][:],
                        v_gath[slab][half].rearrange("(c p) h -> p c h", p=128),
                    )

                # Q^T both slabs -> resident SBUF (fp8 at scale QS)
                for sp in range(QH // 1024):
                    xts = xts_l[sp]
                    base = sp * 1024
                    for hh in range(HC):
                        ps0 = psA.tile([128, 512], F32, tag="psA", name=f"pq0_{sp}_{hh}")
                        ps1 = psA.tile([128, 512], F32, tag="psA", name=f"pq1_{sp}_{hh}")
                        for f in range(FC):
                            lhs = wq_sb[:, f, hh * 128 : (hh + 1) * 128]
                            nc.tensor.matmul(
                                ps0[:], lhs, xts[:, f, 0:512],
                                start=(f == 0), stop=(f == FC - 1),
                            )
                            nc.tensor.matmul(
                                ps1[:], lhs, xts[:, f, 512:1024],
                                start=(f == 0), stop=(f == FC - 1),
                            )
                        bias = bqt[:, hh : hh + 1]
                        nc.scalar.activation(
                            qt_res[:, hh, base : base + 512], ps0[:],
                            AF.Identity, bias=bias, scale=QS,
                        )
                        nc.vector.tensor_scalar(
                            qt_res[:, hh, base + 512 : base + 1024], ps1[:],
                            QS, bias, ALU.mult, ALU.add,
                        )

            # ---------- Phase B: attention, 1024 query rows per tile ----------
            with (
                tc.tile_pool(name="expp", bufs=2) as expp,
                tc.tile_pool(name="pexpp", bufs=4) as pexpp,
                tc.tile_pool(name="obp", bufs=3) as obp,
                tc.tile_pool(name="psS", bufs=3, space="PSUM") as psS,
                tc.tile_pool(name="psO", bufs=2, space="PSUM") as psO,
            ):
                for qt in range(QH // 1024):
                    qbase = qt * 1024
                    # scores + exp + P' quant, one key-chunk at a time
                    exps = {}
                    for pos, k in enumerate(K_ORDER):
                        i, par = pos // 2, pos % 2
                        half, kk = k // 16, k % 16
                        slab, kk8 = kk // 8, kk % 8
                        g = slab * 2 + half
                        if par == 0:
                            exps[i] = expp.tile(
                                [128, 2, 1024], F8, tag=f"e{i}", name=f"e{qt}_{i}"
                            )
                        ps0 = psS.tile([128, 512], F32, tag="psS", name=f"pS0_{qt}_{k}")
                        ps1 = psS.tile([128, 512], F32, tag="psS", name=f"pS1_{qt}_{k}")
                        kslice = slice(kk8 * 128, (kk8 + 1) * 128)
                        for hp in range(HC // 2):
                            nc.tensor.matmul(
                                ps0[:], ktbig[g][:, 2 * hp : 2 * hp + 2, kslice],
                                qt_res[:, 2 * hp : 2 * hp + 2, qbase : qbase + 512],
                                start=(hp == 0), stop=(hp == HC // 2 - 1),
                                perf_mode=DR,
                            )
                        for hp in range(HC // 2):
                            nc.tensor.matmul(
                                ps1[:], ktbig[g][:, 2 * hp : 2 * hp + 2, kslice],
                                qt_res[:, 2 * hp : 2 * hp + 2, qbase + 512 : qbase + 1024],
                                start=(hp == 0), stop=(hp == HC // 2 - 1),
                                perf_mode=DR,
                            )
                        pexp = pexpp.tile([128, 1024], BF16, tag="pexp", name=f"px{qt}_{k}")
                        nc.scalar.activation(pexp[:, 0:512], ps0[:], AF.Exp, scale=EXP_SCALE)
                        nc.scalar.activation(pexp[:, 512:1024], ps1[:], AF.Exp, scale=EXP_SCALE)
                        nc.vector.tensor_scalar(
                            exps[i][:, par, :], pexp[:], -CMEAN, QS,
                            mybir.AluOpType.add, mybir.AluOpType.mult,
                        )
                    # AV: fp8 DoubleRow over key-chunk pairs
                    for q1 in range(8):
                        qo = q1 * 128
                        o0 = psO.tile([128, 512], F32, tag="o0", name=f"o0_{qt}_{q1}")
                        o1 = psO.tile([128, 512], F32, tag="o1", name=f"o1_{qt}_{q1}")
                        osum = psO.tile(
                            [128, 1], F32, tag="osum", bufs=1, name=f"os{qt}_{q1}"
                        )
                        for t in range(16):
                            k0 = K_ORDER[2 * t]
                            half, kk = k0 // 16, k0 % 16
                            slab, j = kk // 8, kk % 8
                            g = slab * 2 + half
                            lhs = exps[t][:, :, qo : qo + 128]
                            first, last = t == 0, t == 15
                            nc.tensor.matmul(
                                o0[:], lhs, vbig[g][:, j : j + 2, 0:512],
                                start=first, stop=last, perf_mode=DR,
                            )
                            nc.tensor.matmul(
                                o1[:], lhs, vbig[g][:, j : j + 2, 512:1024],
                                start=first, stop=last, perf_mode=DR,
                            )
                            nc.tensor.matmul(
                                osum[:], lhs, cones8[:], start=first, stop=last,
                                perf_mode=DR,
                            )
                        den = obp.tile([128, 1], F32, tag="den", name=f"dn{qt}_{q1}")
                        nc.vector.tensor_scalar_add(den[:], osum[:], DEN_ADD)
                        recip = obp.tile([128, 1], F32, tag="recip", name=f"rc{qt}_{q1}")
                        nc.vector.reciprocal(recip[:], den[:])
                        outsb = obp.tile([128, H], BF16, tag="outsb", name=f"ou{qt}_{q1}")
                        acc = obp.tile([128, H], F32, tag="acc", name=f"ac{qt}_{q1}")
                        row = qbase + qo
                        for hf, ps in ((0, o0), (1, o1)):
                            cs = slice(hf * 512, hf * 512 + 512)
                            nc.vector.tensor_tensor(
                                acc[:, cs], ps[:], csfull[:, cs], ALU.add
                            )
                            nc.vector.scalar_tensor_tensor(
                                outsb[:, cs], acc[:, cs], recip[:], bvfull[:, cs],
                                ALU.mult, ALU.add,
                            )
                        nc.sync.dma_start(out_ext[row : row + 128, :], outsb[:])

    nc.compile()
    return nc


def _get_nc():
    global _NC_CACHE
    if _NC_CACHE is None:
        _NC_CACHE = _build_nc()
    return _NC_CACHE


def _make_in_maps(x, Wq, bq, Wk, bk, Wv, bv):
    bf16 = ml_dtypes.bfloat16
    f8 = ml_dtypes.float8_e4m3fn
    def pcmajor(w):  # [F, H] -> [128, FC, H] partition-major
        return np.ascontiguousarray(w.reshape(FC, 128, H).transpose(1, 0, 2))

    wq_b = pcmajor(np.asarray(Wq, np.float32).astype(bf16))
    wk_b32 = np.asarray(Wk, np.float32).astype(bf16).astype(np.float32)
    # [F, H] -> [128, 4 col-pieces, FC, 256]
    wk8 = np.ascontiguousarray(
        (QS * wk_b32).astype(f8).reshape(FC, 128, 4, 256).transpose(1, 2, 0, 3)
    )
    wv_b32 = np.asarray(Wv, np.float32).astype(bf16).astype(np.float32)
    wv8 = pcmajor((QS * wv_b32).astype(f8))
    # activation computes f(scale*x + bias) with scale=QS, so pre-scale biases
    bqt = np.ascontiguousarray(QS * np.asarray(bq, np.float32).reshape(HC, 128).T)
    bkt = np.ascontiguousarray(QS * np.asarray(bk, np.float32).reshape(HC, 128).T)
    bvf = np.ascontiguousarray(
        np.broadcast_to(np.asarray(bv, np.float32).astype(bf16).reshape(1, H), (128, H))
    )
    x = np.asarray(x, np.float32)
    # host-side rank-1 correction row: 256*C*colsum(V0_true) per batch,
    # V0_true = x @ bf16(Wv) (no bias)
    in_maps = []
    for core in range(N_CORES):
        b, h = core // 2, core % 2
        xt_flat = np.ascontiguousarray(x[b, h * QH : (h + 1) * QH].T).astype(bf16)
        x8t_flat = (QS * xt_flat.astype(np.float32)).astype(f8)
        # [F, QH] -> [128, slab 2, FC, 1024] partition-major
        xt = np.ascontiguousarray(
            xt_flat.reshape(FC, 128, 2, 1024).transpose(1, 2, 0, 3)
        )
        x8t = np.ascontiguousarray(
            x8t_flat.reshape(FC, 128, 2, 2, 512).transpose(1, 2, 3, 0, 4)
        )
        csrow = (QS * QS * CMEAN) * (x[b].sum(axis=0) @ wv_b32)
        csf = np.ascontiguousarray(
            np.broadcast_to(csrow.astype(np.float32).astype(bf16).reshape(1, H), (128, H))
        )
        in_maps.append(
            {
                "xt": xt,
                "x8t": x8t,
                "wq": wq_b,
                "wk8": wk8,
                "wv8": wv8,
                "bqt": bqt,
                "bkt": bkt,
                "csf": csf,
                "bvf": bvf,
            }
        )
    return in_maps


def run_on_hw(inputs, trace=False, tmpdir=None):
    """Returns (full_output, BassKernelResults)."""
    nc = _get_nc()
    in_maps = _make_in_maps(**inputs)
    res = run_bass_kernel_spmd(
        nc, in_maps, core_ids=list(range(N_CORES)), trace=trace, tmpdir=tmpdir
    )
    out = np.empty((B, S, H), np.float32)
    for core in range(N_CORES):
        b, h = core // 2, core % 2
        out[b, h * QH : (h + 1) * QH] = res.results[core]["out"].astype(np.float32)
    return out, res


def kernel(x, Wq, bq, Wk, bk, Wv, bv):
    out, _ = run_on_hw(
        {"x": x, "Wq": Wq, "bq": bq, "Wk": Wk, "bk": bk, "Wv": Wv, "bv": bv}
    )
    return out
